# revision 18
# baseline (speedup 1.0000x reference)
"""BotRGCN on 8 Trainium2 NeuronCores (Bass/Tile, SPMD).

v2 strategy (right-multiply refactor): RGCN mean aggregation satisfies
  sum_r mean_{j in N_r(i)} (x_j @ W_r) = sum_r (mean_{j} x_j) @ W_r
so we aggregate raw features x (256 dims) instead of per-relation
transformed features h (2x256 dims).  Per layer:
  1. each core's node-shard x is staged node-major in fp8 and
     AllGathered into a replicated x_full table (half the bytes of the
     baseline h-table AllGather, and fp8 halves it again -> 4x less
     collective traffic),
  2. each core aggregates messages for its destination shard with an
     indexed DMA row-gather from x_full (grouped G dst-tiles per gather
     call to amortize the ~1us SWDGE descriptor-gen cost per call),
     followed by one-hot S-matrix matmuls on the tensor engine that
     perform the segment-sum per (tile, relation); 1/cnt is applied
     after aggregation as a per-partition scalar, so S entries are pure
     0/1 and exact in fp8,
  3. per tile: out = x @ root + agg_0 @ W_0 + agg_1 @ W_1 + bias, with
     agg_r transposed feat-major via the PE; outputs feed the next
     layer both feat-major (SBUF-resident) and node-major (fp8 staged
     for the next AllGather).
S blocks are generated on-device with one DVE/Pool op per block
(iota == dst7), alternating engines to balance load.

Self-contained: hardcodes problem shapes; the host side shards inputs,
builds gather indices + per-block dst7 tables, compiles one SPMD Bass
program and runs it on cores 0-7.
"""
import sys

import numpy as np

for _p in ("/opt/trn_rl_repo",):
    if _p not in sys.path:
        sys.path.insert(0, _p)

import concourse.bacc as bacc
import concourse.mybir as mybir
from concourse import tile

dt = mybir.dt

NCORES = 8
SLOPE = 0.01
DEFAULT_VARIANT = "v2f16tabdveonly"
CHUNK = 32768  # int16 gather-index limit per table slice
G = 3  # dst tiles per gather call
PSPLIT = 24  # tile boundary for piece-major x_full layout / split AllGathers


def _ceil_to(x, m):
    return ((x + m - 1) // m) * m


class Meta:
    pass


def prepare(inputs, ncores=NCORES, node_chunk=448):
    """Shard inputs, build gather indices + per-block S-gen tables."""
    m = Meta()
    N = inputs["des"].shape[0]
    m.N = N
    m.ncores = ncores
    nsh = N // ncores
    assert nsh * ncores == N
    npad = _ceil_to(nsh, 128)
    m.nsh, m.npad = nsh, npad
    T = npad // 128
    m.ntiles = T
    m.node_chunk = node_chunk
    assert npad % node_chunk == 0
    rows = ncores * npad  # x table: [core][node]
    m.rows = rows
    ngr = (rows + CHUNK - 1) // CHUNK
    m.ngr = ngr
    m.chunk_sizes = [min(CHUNK, rows - g * CHUNK) for g in range(ngr)]
    NG = (T + G - 1) // G
    m.NG = NG
    m.G = G

    src = np.asarray(inputs["edge_index"][0], dtype=np.int64)
    dst = np.asarray(inputs["edge_index"][1], dtype=np.int64)
    rel = np.asarray(inputs["edge_type"], dtype=np.int64)

    cnt = np.zeros((2, N), np.int64)
    for r in (0, 1):
        sel = rel == r
        cnt[r] = np.bincount(dst[sel], minlength=N)
    invc = 1.0 / np.maximum(cnt, 1).astype(np.float32)

    core_s, loc_s = src // nsh, src % nsh
    # piece-major table layout: [piece0: cores x tiles 0..PSPLIT) then
    # [piece1: cores x tiles PSPLIT..T) — both AllGather pieces land
    # contiguously in x_full, enabling the split/overlapped collectives.
    pr0 = PSPLIT * 128
    flat = np.where(
        loc_s < pr0,
        core_s * pr0 + loc_s,
        ncores * pr0 + core_s * (npad - pr0) + (loc_s - pr0),
    )
    g_e = flat // CHUNK
    idx16 = flat - g_e * CHUNK

    core_d = dst // nsh
    locd = dst % nsh
    tile_d = locd // 128
    dst7 = locd % 128

    # per-(core, tile, rel, chunk) counts; budget = max over cores,
    # each segment padded to whole 128-row blocks so S-matmul blocks
    # are (tile, rel)-homogeneous within a single gather stream.
    counts = np.zeros((ncores, T, 2, ngr), np.int64)
    np.add.at(counts, (core_d, tile_d, rel, g_e), 1)
    segmax = counts.max(axis=0)  # [T, 2, ngr]
    seg_blk = -(-segmax // 128)  # blocks per (t, r, g)
    m.seg_blk = seg_blk

    # stream layout: for group i: for chunk g: for t in group: for r:
    # segment (t, r, g).  Record per-unit (i,g) offsets and per-block
    # codegen info.
    m.goff = np.zeros((NG, ngr), np.int64)  # idx-stream offset
    m.seg_list = {}  # (i, g) -> [(t, r, off, nblocks)]
    m.gB = np.zeros((NG, ngr), np.int64)  # idx count per unit
    blocks = []  # (i, g, t, r, col, first, last, K)
    seg_off = {}  # (t, r, g) -> idx-stream offset of segment
    off = 0
    col = 0
    for i in range(NG):
        tiles_i = range(i * G, min(T, (i + 1) * G))
        for g in range(ngr):
            m.goff[i, g] = off
            ucol0 = col
            for t in tiles_i:
                for r in (0, 1):
                    nb = int(seg_blk[t, r, g])
                    if nb == 0:
                        continue
                    seg_off[(t, r, g)] = off
                    m.seg_list.setdefault((i, g), []).append(
                        (t, r, off, nb)
                    )
                    for k in range(nb):
                        blocks.append([i, g, t, r, col, 0, 0, 128])
                        off += 128
                        col += 1
            m.gB[i, g] = off - m.goff[i, g]
    m.totidx = int(off)
    m.totnb = int(col)
    # first/last accumulation flags per (t, r)
    byseg = {}
    for b in blocks:
        byseg.setdefault((b[2], b[3]), []).append(b)
    for (t, r), bl in byseg.items():
        bl[0][5] = 1
        bl[-1][6] = 1
    m.blocks = blocks
    m.has_rel = segmax.sum(axis=2) > 0  # [T, 2]

    # stable per-edge segment id in stream order for sorting
    sid_lookup = np.full((T, 2, ngr), -1, np.int64)
    sid = 0
    for i in range(NG):
        tiles_i = range(i * G, min(T, (i + 1) * G))
        for g in range(ngr):
            for t in tiles_i:
                for r in (0, 1):
                    sid_lookup[t, r, g] = sid
                    sid += 1
    nsid = sid
    sid_arr = sid_lookup[tile_d, rel, g_e]
    order = np.lexsort((idx16, sid_arr, core_d))
    o_idx16, o_dst7, o_sid = idx16[order], dst7[order], sid_arr[order]
    key = core_d[order] * nsid + o_sid
    uniq_start = np.searchsorted(key, np.arange(ncores * nsid), "left")
    uniq_end = np.searchsorted(key, np.arange(ncores * nsid), "right")
    # segment stream offsets in sid order
    sid_to_off = np.full((nsid,), -1, np.int64)
    for (t, r, g), o in seg_off.items():
        sid_to_off[sid_lookup[t, r, g]] = o

    des = np.asarray(inputs["des"], np.float32)
    tweet = np.asarray(inputs["tweet"], np.float32)
    nump = np.asarray(inputs["num_prop"], np.float32)
    catp = np.asarray(inputs["cat_prop"], np.float32)

    def shard_T(x, c, kblocks):
        xs = x[c * nsh : (c + 1) * nsh]
        out = np.zeros((kblocks * 128, npad), np.float16)
        out[: x.shape[1], :nsh] = xs.T.astype(np.float16)
        return out.reshape(kblocks, 128, npad)

    def small_T(x, c, d):
        xs = x[c * nsh : (c + 1) * nsh]
        out = np.zeros((d, npad), np.float16)
        out[: x.shape[1], :nsh] = xs.T.astype(np.float16)
        return out

    f16 = np.float16
    w = {}
    w["Wd6"] = np.ascontiguousarray(
        np.asarray(inputs["Wd"], f16).reshape(6, 128, 64)
    )
    w["Wt6"] = np.ascontiguousarray(
        np.asarray(inputs["Wt"], f16).reshape(6, 128, 64)
    )
    w["Wn"] = np.asarray(inputs["Wn"], f16)
    w["Wc"] = np.asarray(inputs["Wc"], f16)
    w["Wi2"] = np.asarray(inputs["Wi"], f16).reshape(2, 128, 256)
    w["W1"] = np.asarray(inputs["rel_w1"], f16).reshape(2, 2, 128, 256)
    w["root1"] = np.asarray(inputs["root_w1"], f16).reshape(2, 128, 256)
    w["W2"] = np.asarray(inputs["rel_w2"], f16).reshape(2, 2, 128, 256)
    w["root2"] = np.asarray(inputs["root_w2"], f16).reshape(2, 128, 256)
    w["Wo1"] = np.asarray(inputs["Wo1"], f16).reshape(2, 128, 256)
    w["Wo2"] = np.asarray(inputs["Wo2"], f16).reshape(2, 128, 2)
    w["bias_a"] = (
        np.concatenate([np.asarray(inputs["bd"]), np.asarray(inputs["bt"])])
        .astype(np.float32)
        .reshape(128, 1)
    )
    w["bias_b"] = (
        np.concatenate([np.asarray(inputs["bn"]), np.asarray(inputs["bc"])])
        .astype(np.float32)
        .reshape(128, 1)
    )
    w["bi_col"] = np.asarray(inputs["bi"], np.float32).reshape(2, 128).T.copy()
    w["bias1_rep"] = np.tile(
        np.asarray(inputs["bias1"], np.float32)[None, :], (128, 1)
    )
    w["bias2_rep"] = np.tile(
        np.asarray(inputs["bias2"], np.float32)[None, :], (128, 1)
    )
    w["bo1_col"] = np.asarray(inputs["bo1"], np.float32).reshape(2, 128).T.copy()
    w["bo2_rep"] = np.tile(
        np.asarray(inputs["bo2"], np.float32)[None, :], (128, 1)
    )
    w["ident"] = np.eye(128, dtype=np.float16)
    w["iota"] = np.tile(np.arange(128, dtype=np.float16)[None, :], (128, 1))

    in_maps = []
    for c in range(ncores):
        idx_all = np.zeros((m.totidx,), np.int16)
        dstb = np.full((m.totnb * 128,), -1.0, np.float32)
        # per-(tile, rel) 1/cnt for this core's dst shard
        invc_t = np.ones((128, 2 * T), np.float32)
        nd = np.arange(128)
        for t in range(T):
            gl = c * nsh + t * 128 + nd
            valid = (t * 128 + nd) < nsh
            for r in (0, 1):
                invc_t[valid, 2 * t + r] = invc[r, gl[valid]]
        for s in range(nsid):
            o = int(sid_to_off[s])
            if o < 0:
                continue
            u = c * nsid + s
            s0, s1 = uniq_start[u], uniq_end[u]
            n = s1 - s0
            if n == 0:
                continue
            idx_all[o : o + n] = o_idx16[s0:s1].astype(np.int16)
            dstb[o : o + n] = o_dst7[s0:s1].astype(np.float32)
        # 16-wrap the idx stream: [128, totidx//16]
        idx_w = np.tile(
            idx_all.reshape(m.totidx // 16, 16).T, (8, 1)
        ).astype(np.int16)
        # wait: wrapping must be per 16 within each contiguous run; the
        # reshape above wraps globally which is only correct because
        # every segment offset and length is a multiple of 128 (>16).
        im = {
            "desT": shard_T(des, c, 6),
            "tweetT": shard_T(tweet, c, 6),
            "numT": small_T(nump, c, 5),
            "catT": small_T(catp, c, 3),
            "idx": idx_w,
            "dstb": np.ascontiguousarray(
                dstb.reshape(m.totnb, 128).T
            ),  # [128, totnb]
            "invct": invc_t,
        }
        im.update(w)
        in_maps.append(im)
    return m, in_maps


# ---------------------------------------------------------------- builder
def build(m, debug=False, repeats=1, variant=DEFAULT_VARIANT, ndev=None):
    nq = 4
    nc = bacc.Bacc(
        "TRN2",
        target_bir_lowering=False,
        debug=debug,
        enable_asserts=True,
        num_devices=m.ncores if ndev is None else ndev,
        num_swdge_queues=nq,
    )
    npad, T = m.npad, m.ntiles
    NCH = m.node_chunk
    NG, ngr = m.NG, m.ngr

    f32, f16, i16 = dt.float32, dt.float16, dt.int16
    f8 = dt.float16 if "f16tab" in variant else dt.float8e4
    ein, eout = "ExternalInput", "ExternalOutput"

    desT = nc.dram_tensor("desT", [6, 128, npad], f16, kind=ein)
    tweetT = nc.dram_tensor("tweetT", [6, 128, npad], f16, kind=ein)
    numT = nc.dram_tensor("numT", [5, npad], f16, kind=ein)
    catT = nc.dram_tensor("catT", [3, npad], f16, kind=ein)
    idx_ext = nc.dram_tensor("idx", [128, m.totidx // 16], i16, kind=ein)
    dst_ext = nc.dram_tensor("dstb", [128, m.totnb], f32, kind=ein)
    invct_ext = nc.dram_tensor("invct", [128, 2 * T], f32, kind=ein)
    Wd6 = nc.dram_tensor("Wd6", [6, 128, 64], f16, kind=ein)
    Wt6 = nc.dram_tensor("Wt6", [6, 128, 64], f16, kind=ein)
    Wn = nc.dram_tensor("Wn", [5, 64], f16, kind=ein)
    Wc = nc.dram_tensor("Wc", [3, 64], f16, kind=ein)
    Wi2 = nc.dram_tensor("Wi2", [2, 128, 256], f16, kind=ein)
    W1 = nc.dram_tensor("W1", [2, 2, 128, 256], f16, kind=ein)
    root1 = nc.dram_tensor("root1", [2, 128, 256], f16, kind=ein)
    W2 = nc.dram_tensor("W2", [2, 2, 128, 256], f16, kind=ein)
    root2 = nc.dram_tensor("root2", [2, 128, 256], f16, kind=ein)
    Wo1 = nc.dram_tensor("Wo1", [2, 128, 256], f16, kind=ein)
    Wo2 = nc.dram_tensor("Wo2", [2, 128, 2], f16, kind=ein)
    bias_a = nc.dram_tensor("bias_a", [128, 1], f32, kind=ein)
    bias_b = nc.dram_tensor("bias_b", [128, 1], f32, kind=ein)
    bi_col = nc.dram_tensor("bi_col", [128, 2], f32, kind=ein)
    bias1_rep = nc.dram_tensor("bias1_rep", [128, 256], f32, kind=ein)
    bias2_rep = nc.dram_tensor("bias2_rep", [128, 256], f32, kind=ein)
    bo1_col = nc.dram_tensor("bo1_col", [128, 2], f32, kind=ein)
    bo2_rep = nc.dram_tensor("bo2_rep", [128, 2], f32, kind=ein)
    ident = nc.dram_tensor("ident", [128, 128], f16, kind=ein)
    iota_ext = nc.dram_tensor("iota", [128, 128], f16, kind=ein)
    out_ext = nc.dram_tensor("out", [npad, 2], f32, kind=eout)

    x1_loc = nc.dram_tensor("x1_loc", [npad, 256], f8)
    x1_full = nc.dram_tensor("x1_full", [m.rows, 256], f8, addr_space="Shared")
    x2_loc = nc.dram_tensor("x2_loc", [npad, 256], f8)
    x2_full = nc.dram_tensor("x2_full", [m.rows, 256], f8, addr_space="Shared")

    def AG(loc, full):
        if "no_ag" in variant:
            nc.sync.dma_start(full.ap()[0 : loc.shape[0], :], loc[:])
        else:
            nc.gpsimd.collective_compute(
                "AllGather",
                mybir.AluOpType.bypass,
                ins=[loc[:]],
                outs=[full[:]],
                replica_groups=[list(range(m.ncores))],
            )

    def AG_piece(loc, full, ta, tb):
        # piece-major layout: rows of tiles [ta, tb) from all cores land
        # contiguously at full[ncores*ta*128 ...]
        o0 = m.ncores * ta * 128
        o1 = o0 + m.ncores * (tb - ta) * 128
        nc.gpsimd.collective_compute(
            "AllGather",
            mybir.AluOpType.bypass,
            ins=[loc.ap()[ta * 128 : tb * 128, :]],
            outs=[full.ap()[o0:o1, :]],
            replica_groups=[list(range(m.ncores))],
        )

    def lrelu_from(pool, dst_ap, src_ap, bias_ap, shape):
        t0 = pool.tile(shape, f16, tag="lr0", name="lr0")
        nc.scalar.activation(
            t0[:], src_ap, mybir.ActivationFunctionType.Identity, bias=bias_ap
        )
        t1 = pool.tile(shape, f16, tag="lr1", name="lr1")
        nc.vector.tensor_scalar_mul(t1[:], t0[:], SLOPE)
        nc.vector.tensor_max(dst_ap, t0[:], t1[:])

    # group blocks by (group i) for codegen
    blocks_by_grp = [[] for _ in range(NG)]
    for b in m.blocks:
        blocks_by_grp[b[0]].append(b)
    nb_grp = [len(bl) for bl in blocks_by_grp]
    nb_grp_max = max(nb_grp)
    nbg_max = [int(max(m.gB[:, g]) // 128) for g in range(ngr)]

    with tile.TileContext(nc) as tc:
        with (
            tc.tile_pool(name="wpool", bufs=1) as wp,
            tc.tile_pool(name="xres", bufs=1) as xres,
        ):
            wd_sb = wp.tile([128, 6, 64], f16)
            nc.sync.dma_start(wd_sb[:], _pmaj(Wd6))
            wt_sb = wp.tile([128, 6, 64], f16)
            nc.sync.dma_start(wt_sb[:], _pmaj(Wt6))
            wn_sb = wp.tile([5, 64], f16)
            nc.sync.dma_start(wn_sb[:], Wn[:])
            wc_sb = wp.tile([3, 64], f16)
            nc.sync.dma_start(wc_sb[:], Wc[:])
            wi_sb = wp.tile([128, 2, 256], f16)
            nc.sync.dma_start(wi_sb[:], _pmaj(Wi2))
            w1_sb = wp.tile([128, 4, 256], f16)
            nc.sync.dma_start(w1_sb[:], W1.ap().rearrange("r k p m -> p (r k) m"))
            r1_sb = wp.tile([128, 2, 256], f16)
            nc.sync.dma_start(r1_sb[:], _pmaj(root1))
            w2_sb = wp.tile([128, 4, 256], f16)
            nc.sync.dma_start(w2_sb[:], W2.ap().rearrange("r k p m -> p (r k) m"))
            r2_sb = wp.tile([128, 2, 256], f16)
            nc.sync.dma_start(r2_sb[:], _pmaj(root2))
            wo1_sb = wp.tile([128, 2, 256], f16)
            nc.sync.dma_start(wo1_sb[:], _pmaj(Wo1))
            wo2_sb = wp.tile([128, 2, 2], f16)
            nc.sync.dma_start(wo2_sb[:], _pmaj(Wo2))
            ba_sb = wp.tile([128, 1], f32)
            nc.sync.dma_start(ba_sb[:], bias_a[:])
            bb_sb = wp.tile([128, 1], f32)
            nc.sync.dma_start(bb_sb[:], bias_b[:])
            bi_sb = wp.tile([128, 2], f32)
            nc.sync.dma_start(bi_sb[:], bi_col[:])
            b1_sb = wp.tile([128, 256], f32)
            nc.sync.dma_start(b1_sb[:], bias1_rep[:])
            b2_sb = wp.tile([128, 256], f32)
            nc.sync.dma_start(b2_sb[:], bias2_rep[:])
            bo1_sb = wp.tile([128, 2], f32)
            nc.sync.dma_start(bo1_sb[:], bo1_col[:])
            bo2_sb = wp.tile([128, 2], f32)
            nc.sync.dma_start(bo2_sb[:], bo2_rep[:])
            id_sb = wp.tile([128, 128], f16)
            nc.sync.dma_start(id_sb[:], ident[:])
            io_sb = wp.tile([128, 128], f16)
            nc.sync.dma_start(io_sb[:], iota_ext[:])
            idx_sb = wp.tile([128, m.totidx // 16], i16)
            nc.sync.dma_start(idx_sb[:], idx_ext[:])
            dst_sb = wp.tile([128, m.totnb], f32)
            nc.sync.dma_start(dst_sb[:], dst_ext[:])
            invct_sb = wp.tile([128, 2 * T], f32)
            nc.sync.dma_start(invct_sb[:], invct_ext[:])

            x1a = xres.tile([128, npad], f16, tag="x1a")
            x1b = xres.tile([128, npad], f16, tag="x1b")
            x2a = xres.tile([128, npad], f16, tag="x2a")
            x2b = xres.tile([128, npad], f16, tag="x2b")
            out_stage = xres.tile([128, T, 2], f32, tag="outst")

            def whole_body():
                # -------- phase 0: feature pipeline -> x1 (feat-major) ----
                with (
                    tc.tile_pool(name="p0", bufs=2) as p0,
                    tc.tile_pool(name="p0ps", bufs=2, space="PSUM") as p0ps,
                ):
                    for c0 in range(0, npad, NCH):
                        dsb = p0.tile([128, 6, NCH], f16, tag="des", name="dsb")
                        nc.sync.dma_start(
                            dsb[:],
                            desT.ap()[:, :, c0 : c0 + NCH].rearrange(
                                "k p n -> p k n"
                            ),
                        )
                        tsb = p0.tile([128, 6, NCH], f16, tag="tw", name="tsb")
                        nc.sync.dma_start(
                            tsb[:],
                            tweetT.ap()[:, :, c0 : c0 + NCH].rearrange(
                                "k p n -> p k n"
                            ),
                        )
                        nsb = p0.tile([5, NCH], f16, tag="np", name="nsb")
                        nc.sync.dma_start(nsb[:], numT.ap()[:, c0 : c0 + NCH])
                        csb = p0.tile([3, NCH], f16, tag="cp", name="csb")
                        nc.sync.dma_start(csb[:], catT.ap()[:, c0 : c0 + NCH])

                        ps_a = p0ps.tile([128, NCH], f32, tag="psa", name="ps_a")
                        for k in range(6):
                            nc.tensor.matmul(
                                ps_a[0:64, :],
                                wd_sb[:, k, :],
                                dsb[:, k, :],
                                start=(k == 0),
                                stop=(k == 5),
                            )
                        for k in range(6):
                            nc.tensor.matmul(
                                ps_a[64:128, :],
                                wt_sb[:, k, :],
                                tsb[:, k, :],
                                start=(k == 0),
                                stop=(k == 5),
                                tile_position=(0, 64),
                            )
                        ps_b = p0ps.tile([128, NCH], f32, tag="psb", name="ps_b")
                        nc.tensor.matmul(
                            ps_b[0:64, :], wn_sb[:], nsb[:], start=True, stop=True
                        )
                        nc.tensor.matmul(
                            ps_b[64:128, :],
                            wc_sb[:],
                            csb[:],
                            start=True,
                            stop=True,
                            tile_position=(0, 64),
                        )
                        x0a = p0.tile([128, NCH], f16, tag="x0a", name="x0a")
                        lrelu_from(p0, x0a[:], ps_a[:], ba_sb[:], [128, NCH])
                        x0b = p0.tile([128, NCH], f16, tag="x0b", name="x0b")
                        lrelu_from(p0, x0b[:], ps_b[:], bb_sb[:], [128, NCH])

                        for h, xdst in ((0, x1a), (1, x1b)):
                            ps_x = p0ps.tile(
                                [128, NCH], f32, tag="psx", name="ps_x"
                            )
                            nc.tensor.matmul(
                                ps_x[:],
                                wi_sb[:, 0, h * 128 : (h + 1) * 128],
                                x0a[:],
                                start=True,
                                stop=False,
                            )
                            nc.tensor.matmul(
                                ps_x[:],
                                wi_sb[:, 1, h * 128 : (h + 1) * 128],
                                x0b[:],
                                start=False,
                                stop=True,
                            )
                            lrelu_from(
                                p0,
                                xdst[:, c0 : c0 + NCH],
                                ps_x[:],
                                bi_sb[:, h : h + 1],
                                [128, NCH],
                            )

                # -------- phase 0b: stage x1 node-major fp8 + AG ----------
                with (
                    tc.tile_pool(name="st", bufs=3) as stp,
                    tc.tile_pool(name="stps", bufs=4, space="PSUM") as stps,
                ):
                    for t0 in range(0, T, 4):
                        nt = min(4, T - t0)
                        stg = stp.tile([128, 4, 256], f8, tag="stg", name="stg")
                        for j in range(nt):
                            t = t0 + j
                            ts = slice(t * 128, (t + 1) * 128)
                            for h, xsrc in ((0, x1a), (1, x1b)):
                                pt = stps.tile(
                                    [128, 128], f16, tag="pt", name="pt"
                                )
                                nc.tensor.transpose(
                                    pt[:], xsrc[:, ts], id_sb[:]
                                )
                                nc.scalar.activation(
                                    stg[:, j, h * 128 : (h + 1) * 128],
                                    pt[:],
                                    mybir.ActivationFunctionType.Identity,
                                )
                        nc.sync.dma_start(
                            x1_loc.ap()[
                                t0 * 128 : (t0 + nt) * 128, :
                            ].rearrange("(b p) f -> p b f", p=128),
                            stg[:, 0:nt, :],
                        )
                        if t0 + nt == PSPLIT:
                            AG_piece(x1_loc, x1_full, 0, PSPLIT)
                AG_piece(x1_loc, x1_full, PSPLIT, T)

                # -------- RGCN layers -------------------------------------
                def rgcn_layer(xa, xb, r_sb, w_sb, b_sb, x_full, out_cb, lp, lps):
                    for i in range(NG):
                        tiles_i = list(range(i * G, min(T, (i + 1) * G)))
                        msgs = {}
                        for g in range(ngr):
                            B = int(m.gB[i, g])
                            if B == 0:
                                continue
                            nbg = B // 128
                            gb = g * CHUNK
                            gs = m.chunk_sizes[g]
                            msg = lp.tile(
                                [128, nbg_max[g], 256],
                                f8,
                                tag=f"msg{g}",
                                name=f"msg{g}",
                                bufs=2,
                            )
                            o = int(m.goff[i, g])
                            if "smallg" in variant:
                                for t, r, so, nb in m.seg_list[(i, g)]:
                                    b0 = (so - o) // 128
                                    nc.gpsimd.dma_gather(
                                        msg[:, b0 : b0 + nb, :],
                                        x_full.ap()[gb : gb + gs, :],
                                        idx_sb[
                                            :, so // 16 : (so + nb * 128) // 16
                                        ],
                                        num_idxs=nb * 128,
                                        num_idxs_reg=nb * 128,
                                        elem_size=256,
                                        single_packet=False,
                                        queue_num=((b0 + g) % nq),
                                    )
                            else:
                                nc.gpsimd.dma_gather(
                                    msg[:, 0:nbg, :],
                                    x_full.ap()[gb : gb + gs, :],
                                    idx_sb[:, o // 16 : (o + B) // 16],
                                    num_idxs=B,
                                    num_idxs_reg=B,
                                    elem_size=256,
                                    single_packet=False,
                                    queue_num=(g % nq),
                                )
                            msgs[g] = msg
                        # S blocks for this group (alternate DVE / Pool)
                        ssb = lp.tile(
                            [128, nb_grp_max, 128],
                            f8,
                            tag="ssb",
                            name="ssb",
                            bufs=2,
                        )
                        for j, b in enumerate(blocks_by_grp[i]):
                            col = b[4]
                            eng = (
                                nc.vector
                                if ("dveonly" in variant or j % 2 == 0)
                                else nc.gpsimd
                            )
                            eng.tensor_scalar(
                                ssb[:, j, :],
                                io_sb[:],
                                dst_sb[:, col : col + 1],
                                None,
                                op0=mybir.AluOpType.is_equal,
                            )
                        # aggregation matmuls, per-(tile, rel) PSUM bank;
                        # blocks of one (t, r) stream contiguously across
                        # chunks so only one accumulation group is open per
                        # bank at a time.
                        jmap = {
                            b[4]: j for j, b in enumerate(blocks_by_grp[i])
                        }
                        bytr = {}
                        for b in blocks_by_grp[i]:
                            bytr.setdefault((b[2], b[3]), []).append(b)
                        po_t = {}
                        for (t, r), bl in bytr.items():
                            po = lps.tile(
                                [128, 256], f32, tag="po", name="po", bufs=3
                            )
                            po_t[(t, r)] = po
                            for b in bl:
                                _, g, _, _, col, first, last, K = b
                                bg = col - int(m.goff[i, g]) // 128
                                nc.tensor.matmul(
                                    po[:],
                                    ssb[:, jmap[col], :],
                                    msgs[g][:, bg, :],
                                    start=bool(first),
                                    stop=bool(last),
                                )
                        # epilogue per tile
                        for t in tiles_i:
                            ts = slice(t * 128, (t + 1) * 128)
                            sg = lp.tile(
                                [128, 512], f16, tag="sg", name="sg"
                            )
                            for r in (0, 1):
                                if m.has_rel[t, r]:
                                    nc.vector.tensor_scalar(
                                        sg[:, r * 256 : (r + 1) * 256],
                                        po_t[(t, r)][:],
                                        invct_sb[:, 2 * t + r : 2 * t + r + 1],
                                        None,
                                        op0=mybir.AluOpType.mult,
                                    )
                            aggT = lp.tile(
                                [128, 4, 128], f16, tag="aggT", name="aggT"
                            )
                            for r in (0, 1):
                                if not m.has_rel[t, r]:
                                    continue
                                pt = lps.tile(
                                    [128, 2, 128], f16, tag="pt", name="pt"
                                )
                                for k in (0, 1):
                                    nc.tensor.transpose(
                                        pt[:, k, :],
                                        sg[:, r * 256 + k * 128 : r * 256 + (k + 1) * 128],
                                        id_sb[:],
                                    )
                                nc.scalar.activation(
                                    aggT[:, 2 * r : 2 * r + 2, :],
                                    pt[:],
                                    mybir.ActivationFunctionType.Identity,
                                )
                            po2 = lps.tile(
                                [128, 256], f32, tag="po2", name="po2"
                            )
                            nc.tensor.matmul(
                                po2[:], xa[:, ts], r_sb[:, 0, :],
                                start=True, stop=False,
                            )
                            nmm = 2 + 2 * int(m.has_rel[t, 0]) + 2 * int(
                                m.has_rel[t, 1]
                            )
                            done = 2
                            nc.tensor.matmul(
                                po2[:], xb[:, ts], r_sb[:, 1, :],
                                start=False, stop=(done == nmm),
                            )
                            for r in (0, 1):
                                if not m.has_rel[t, r]:
                                    continue
                                for k in (0, 1):
                                    done += 1
                                    nc.tensor.matmul(
                                        po2[:],
                                        aggT[:, 2 * r + k, :],
                                        w_sb[:, 2 * r + k, :],
                                        start=False,
                                        stop=(done == nmm),
                                    )
                            osb = lp.tile([128, 256], f16, tag="osb", name="osb")
                            nc.vector.tensor_add(osb[:], po2[:], b_sb[:])
                            out_cb(t, ts, osb, lp, lps)

                x2_stage = [None]

                def l1_out(t, ts, osb, lp, lps):
                    # feat-major for layer 2 compute
                    pt = lps.tile([128, 2, 128], f16, tag="pt", name="ptT")
                    for h in range(2):
                        nc.tensor.transpose(
                            pt[:, h, :], osb[:, h * 128 : (h + 1) * 128], id_sb[:]
                        )
                    for h, xdst in ((0, x2a), (1, x2b)):
                        nc.vector.tensor_copy(xdst[:, ts], pt[:, h, :])
                    # node-major fp8 staging for AG2
                    j = t % 4
                    if j == 0:
                        x2_stage[0] = lp.tile(
                            [128, 4, 256], f8, tag="x2st", name="x2st"
                        )
                    nc.scalar.activation(
                        x2_stage[0][:, j, :],
                        osb[:],
                        mybir.ActivationFunctionType.Identity,
                    )
                    if j == 3 or t == T - 1:
                        t0 = t - j
                        nt = j + 1
                        nc.sync.dma_start(
                            x2_loc.ap()[
                                t0 * 128 : (t0 + nt) * 128, :
                            ].rearrange("(b p) f -> p b f", p=128),
                            x2_stage[0][:, 0:nt, :],
                        )
                        if t + 1 == PSPLIT:
                            AG_piece(x2_loc, x2_full, 0, PSPLIT)

                def l2_out(t, ts, osb, lp, lps):
                    o2t = lp.tile([128, 2, 128], f16, tag="o2t", name="o2t")
                    pt = lps.tile([128, 2, 128], f16, tag="pt", name="pt2")
                    for h in range(2):
                        nc.tensor.transpose(
                            pt[:, h, :], osb[:, h * 128 : (h + 1) * 128], id_sb[:]
                        )
                    nc.vector.tensor_copy(o2t[:], pt[:])
                    ht = lp.tile([128, 2, 128], f16, tag="ht", name="ht")
                    for h in range(2):
                        phd = lps.tile([128, 256], f32, tag="po2", name="phd")
                        nc.tensor.matmul(
                            phd[:, 0:128],
                            wo1_sb[:, 0, h * 128 : (h + 1) * 128],
                            o2t[:, 0, :],
                            start=True,
                            stop=False,
                        )
                        nc.tensor.matmul(
                            phd[:, 0:128],
                            wo1_sb[:, 1, h * 128 : (h + 1) * 128],
                            o2t[:, 1, :],
                            start=False,
                            stop=True,
                        )
                        lrelu_from(
                            lp, ht[:, h, :], phd[:, 0:128], bo1_sb[:, h : h + 1],
                            [128, 128],
                        )
                    pf = lps.tile([128, 256], f32, tag="po2", name="pf")
                    nc.tensor.matmul(
                        pf[:, 0:2], ht[:, 0, :], wo2_sb[:, 0, :],
                        start=True, stop=False,
                    )
                    nc.tensor.matmul(
                        pf[:, 0:2], ht[:, 1, :], wo2_sb[:, 1, :],
                        start=False, stop=True,
                    )
                    nc.vector.tensor_add(
                        out_stage[:, t, :], pf[:, 0:2], bo2_sb[:]
                    )
                    if t == T - 1:
                        nc.sync.dma_start(
                            out_ext.ap().rearrange("(b p) f -> p b f", p=128),
                            out_stage[:],
                        )

                with (
                    tc.tile_pool(name="lyr", bufs=2) as lp,
                    tc.tile_pool(name="lyrps", bufs=2, space="PSUM") as lps,
                ):
                    rgcn_layer(
                        x1a, x1b, r1_sb, w1_sb, b1_sb, x1_full, l1_out, lp, lps
                    )
                    AG_piece(x2_loc, x2_full, PSPLIT, T)
                    rgcn_layer(
                        x2a, x2b, r2_sb, w2_sb, b2_sb, x2_full, l2_out, lp, lps
                    )

            for _rep in range(repeats):
                whole_body()

    nc.compile()
    return nc


def _pmaj(t):
    """DRAM tensor [a, 128, b] viewed partition-major [128, a, b]."""
    return t.ap().rearrange("a p b -> p a b")


# ---------------------------------------------------------------- entry
def kernel(**inputs):
    meta, in_maps = prepare(inputs)
    nc = build(meta, variant=DEFAULT_VARIANT)
    from concourse.bass_utils import run_bass_kernel_spmd

    res = run_bass_kernel_spmd(
        nc, in_maps, core_ids=list(range(meta.ncores))
    ).results
    out = np.concatenate(
        [res[c]["out"][: meta.nsh] for c in range(meta.ncores)], axis=0
    )
    return out.astype(np.float32)
